# revision 3
# baseline (speedup 1.0000x reference)
"""Self-contained Trainium2 Bass kernel for CausalWanSelfAttention.

Sharding: 8 cores = 2 sequence-halves x 4 head-groups (3 heads each).
Per-core work: QKV projections for (its 780 tokens x its 384 dims), RMS
normalizers completed via a tiny AllReduce of partial sum-of-squares,
rope on device, attention of (780 queries x 3 heads) against the full
7800-position KV (past KV fed per-head from host; new KV exchanged with
the partner half via AllGather), then a partial O-projection. The host
sums the 4 head-group partials of y and stitches rk/v slices.

Matmuls run in float32r (11-bit mantissa, fp32 accumulate). All DRAM
operands that feed matmuls are pre-rounded on the host so DMA'd tiles
are canonical fp32r values.
"""

import sys

if "/opt/trn_rl_repo" not in sys.path:
    sys.path.insert(0, "/opt/trn_rl_repo")

import math

import numpy as np

import concourse.bass as bass
import concourse.mybir as mybir
import concourse.tile as tile
from concourse import bacc
from concourse.bass_utils import run_bass_kernel_spmd
from concourse.masks import make_identity

F32 = mybir.dt.float32
F32R = mybir.dt.float32r
AF = mybir.ActivationFunctionType
ALU = mybir.AluOpType

B, S, D, N, HD = 1, 1560, 1536, 12, 128
F, H, W = 1, 30, 52
PAST = 6240
TOT = PAST + S
EPS = 1e-6
NC_ = 8
SH = S // 2            # 780 tokens per sequence half
HG = 3                 # heads per group
HGD = HG * HD          # 384 dims per group
KD = D // 128          # 12 contraction tiles for projections
SCALE = 1.0 / math.sqrt(HD)

# token tiles within a half: 6 x 128 + 12
TOK = [(t, min(128, SH - t)) for t in range(0, SH, 128)]
# query blocks (moving-operand split, both >=256 for fp32r full rate)
QB = [(0, 512), (512, 268)]
# past kpos tiles: 48 x 128 + 96
PTILES = [(t, min(128, PAST - t)) for t in range(0, PAST, 128)]
# new kpos tiles per gathered half-block
NTILES = [(t, min(128, SH - t)) for t in range(0, SH, 128)]

_BUILT = {}


def _round_f32r(x):
    xi = np.ascontiguousarray(x, dtype=np.float32).view(np.uint32)
    r = (xi + np.uint32(0x7FF) + ((xi >> np.uint32(12)) & np.uint32(1))) & np.uint32(
        0xFFFFF000
    )
    return r.view(np.float32)


def _build():
    if "nc" in _BUILT:
        return _BUILT["nc"]
    nc = bacc.Bacc("TRN2", target_bir_lowering=False, debug=False, num_devices=NC_)

    # ---- I/O ----
    xT = nc.dram_tensor("xT", [D, SH], F32, kind="ExternalInput")
    wqT = nc.dram_tensor("wqT", [D, HGD], F32, kind="ExternalInput")
    wkT = nc.dram_tensor("wkT", [D, HGD], F32, kind="ExternalInput")
    wvT = nc.dram_tensor("wvT", [D, HGD], F32, kind="ExternalInput")
    woT = nc.dram_tensor("woT", [HGD, D], F32, kind="ExternalInput")
    b3 = nc.dram_tensor("b3", [3, HGD], F32, kind="ExternalInput")
    cgq = nc.dram_tensor("cgq", [SH, HGD], F32, kind="ExternalInput")
    sgq = nc.dram_tensor("sgq", [SH, HGD], F32, kind="ExternalInput")
    cgk = nc.dram_tensor("cgk", [SH, HGD], F32, kind="ExternalInput")
    sgk = nc.dram_tensor("sgk", [SH, HGD], F32, kind="ExternalInput")
    pastkT = nc.dram_tensor("pastkT", [HGD, PAST], F32, kind="ExternalInput")
    pastv = nc.dram_tensor("pastv", [PAST, HGD], F32, kind="ExternalInput")

    y_part = nc.dram_tensor("y_part", [SH, D], F32, kind="ExternalOutput")
    rk_half = nc.dram_tensor("rk_half", [SH, HGD], F32, kind="ExternalOutput")
    v_half = nc.dram_tensor("v_half", [SH, HGD], F32, kind="ExternalOutput")

    # ---- DRAM scratch for collectives ----
    ssk_in = nc.dram_tensor("ssk_in", [SH], F32)
    ssk_out = nc.dram_tensor("ssk_out", [SH], F32)
    ssq_in = nc.dram_tensor("ssq_in", [SH], F32)
    ssq_out = nc.dram_tensor("ssq_out", [SH], F32)
    kT_in = nc.dram_tensor("kT_in", [HGD, SH], F32)
    kT_gath = nc.dram_tensor("kT_gath", [2, HGD, SH], F32)
    v_in = nc.dram_tensor("v_in", [SH, HGD], F32)
    v_gath = nc.dram_tensor("v_gath", [2, SH, HGD], F32)

    G4 = [[0, 1, 2, 3], [4, 5, 6, 7]]
    G2 = [[0, 4], [1, 5], [2, 6], [3, 7]]

    with tile.TileContext(nc) as tc:
        with (
            tc.tile_pool(name="const", bufs=1) as constp,
            tc.tile_pool(name="resident", bufs=1) as resp,
        ):
            ident = constp.tile([128, 128], F32, tag="ident")
            make_identity(nc, ident)
            ones_f = constp.tile([128, 1], F32, tag="ones_f")
            nc.vector.memset(ones_f[:], 1.0)
            ones_col = constp.tile([128, 1], F32R, tag="ones_col")
            nc.vector.tensor_copy(ones_col[:], ones_f[:])
            eps_t = constp.tile([128, 1], F32, tag="eps_t")
            nc.vector.memset(eps_t[:], EPS)
            onesr_f = constp.tile([1, 128], F32, tag="onesr_f")
            nc.vector.memset(onesr_f[:], 1.0)
            ones_row = constp.tile([1, 128], F32R, tag="ones_row")
            nc.vector.tensor_copy(ones_row[:], onesr_f[:])

            # resident across stages
            qT = [resp.tile([128, SH], F32R, tag=f"qT{h}", name=f"qT{h}") for h in range(HG)]
            o_sb = [resp.tile([128, SH], F32R, tag=f"osb{h}", name=f"osb{h}") for h in range(HG)]

            # ================= stage P: projections =================
            with (
                tc.tile_pool(name="xw", bufs=1) as xwp,
                tc.tile_pool(name="projps", bufs=3, space="PSUM") as pps,
                tc.tile_pool(name="ptrans", bufs=2, space="PSUM") as tps,
                tc.tile_pool(name="pwork", bufs=3) as wkp,
                tc.tile_pool(name="praw", bufs=1) as rawp,
                tc.tile_pool(name="small", bufs=4) as smp,
            ):
                xt = []
                for kt in range(KD):
                    t = xwp.tile([128, SH], F32R, tag=f"x{kt}", name=f"x{kt}")
                    nc.sync.dma_start(t[:], xT[kt * 128 : (kt + 1) * 128, :].bitcast(F32R))
                    xt.append(t)
                wt = {}
                for nm, dram in (("k", wkT), ("q", wqT), ("v", wvT)):
                    for kt in range(KD):
                        t = xwp.tile([128, HGD], F32R, tag=f"w{nm}{kt}", name=f"w{nm}{kt}")
                        nc.sync.dma_start(
                            t[:], dram[kt * 128 : (kt + 1) * 128, :].bitcast(F32R)
                        )
                        wt[nm, kt] = t
                bias = {}
                for pi, nm in enumerate(("q", "k", "v")):
                    t = xwp.tile([1, HGD], F32R, tag=f"b{nm}", name=f"b{nm}")
                    nc.sync.dma_start(t[:], b3[pi : pi + 1, :].bitcast(F32R))
                    bias[nm] = t

                raws = {}
                # K first, then Q, then V — hides the ss collectives
                for nm, ss_dram in (("k", ssk_in), ("q", ssq_in), ("v", None)):
                    for ti, (t0, tw) in enumerate(TOK):
                        ps = pps.tile([128, HGD], F32, tag="projps")
                        for kt in range(KD):
                            nc.tensor.matmul(
                                ps[:tw, :],
                                xt[kt][:, t0 : t0 + tw],
                                wt[nm, kt][:, :],
                                start=(kt == 0),
                                stop=False,
                            )
                        nc.tensor.matmul(
                            ps[:tw, :],
                            ones_row[:1, :tw],
                            bias[nm][:1, :],
                            start=False,
                            stop=True,
                        )
                        if nm in ("q", "k"):
                            sq = wkp.tile([128, HGD], F32, tag="sqscratch")
                            ss = smp.tile([128, 1], F32, tag="ss")
                            nc.scalar.activation(
                                sq[:tw, :], ps[:tw, :], AF.Square, accum_out=ss[:tw, :]
                            )
                            nc.sync.dma_start(ss_dram[t0 : t0 + tw], ss[:tw, 0:1])
                            raw = rawp.tile([128, HGD], F32, tag=f"raw_{nm}_{ti}", name=f"raw_{nm}_{ti}")
                            nc.vector.tensor_copy(raw[:tw, :], ps[:tw, :])
                            raws[nm, ti] = raw
                        else:
                            vrow = wkp.tile([128, HGD], F32, tag="vrow")
                            nc.vector.tensor_copy(vrow[:tw, :], ps[:tw, :])
                            nc.sync.dma_start(v_half[t0 : t0 + tw, :], vrow[:tw, :])
                            vr = wkp.tile([128, HGD], F32R, tag="vround")
                            nc.vector.tensor_copy(vr[:tw, :], vrow[:tw, :])
                            nc.sync.dma_start(
                                v_in[t0 : t0 + tw, :], vr[:tw, :].bitcast(F32)
                            )
                    if nm == "k":
                        nc.gpsimd.collective_compute(
                            "AllReduce", ALU.add, replica_groups=G4,
                            ins=[ssk_in[:]], outs=[ssk_out[:]],
                        )
                    elif nm == "q":
                        nc.gpsimd.collective_compute(
                            "AllReduce", ALU.add, replica_groups=G4,
                            ins=[ssq_in[:]], outs=[ssq_out[:]],
                        )

                # v AllGather (exchange halves)
                nc.gpsimd.collective_compute(
                    "AllGather", ALU.bypass, replica_groups=G2,
                    ins=[v_in[:]], outs=[v_gath[:]],
                )

                # ---- rms + rope + transpose for k then q ----
                kTm = [resp.tile([128, SH], F32R, tag=f"kTm{h}", name=f"kTm{h}") for h in range(HG)]
                for nm, ss_dram, cg_d, sg_d in (
                    ("k", ssk_out, cgk, sgk),
                    ("q", ssq_out, cgq, sgq),
                ):
                    for ti, (t0, tw) in enumerate(TOK):
                        sst = smp.tile([128, 1], F32, tag="sst")
                        nc.sync.dma_start(sst[:tw, 0:1], ss_dram[t0 : t0 + tw])
                        sq2 = smp.tile([128, 1], F32, tag="sq2")
                        nc.scalar.activation(
                            sq2[:tw, :], sst[:tw, :], AF.Sqrt, scale=1.0 / D, bias=eps_t[:tw, :]
                        )
                        rnorm = smp.tile([128, 1], F32, tag="rnorm")
                        nc.vector.reciprocal(rnorm[:tw, :], sq2[:tw, :])

                        cg_t = wkp.tile([128, HGD], F32, tag="cg")
                        nc.sync.dma_start(cg_t[:tw, :], cg_d[t0 : t0 + tw, :])
                        sg_t = wkp.tile([128, HGD], F32, tag="sg")
                        nc.sync.dma_start(sg_t[:tw, :], sg_d[t0 : t0 + tw, :])

                        raw = raws[nm, ti]
                        rv = raw[:tw, :].rearrange("p (c two) -> p c two", two=2)
                        swp = wkp.tile([128, HGD], F32, tag="swp")
                        sv = swp[:tw, :].rearrange("p (c two) -> p c two", two=2)
                        nc.vector.tensor_copy(sv[:, :, 0:1], rv[:, :, 1:2])
                        nc.vector.tensor_copy(sv[:, :, 1:2], rv[:, :, 0:1])
                        t1 = wkp.tile([128, HGD], F32, tag="ropet1")
                        nc.vector.scalar_tensor_tensor(
                            t1[:tw, :], raw[:tw, :], rnorm[:tw, :], cg_t[:tw, :],
                            ALU.mult, ALU.mult,
                        )
                        rows = wkp.tile([128, HGD], F32, tag="rows")
                        nc.vector.scalar_tensor_tensor(
                            rows[:tw, :], swp[:tw, :], rnorm[:tw, :], sg_t[:tw, :],
                            ALU.mult, ALU.mult,
                        )
                        nc.vector.tensor_add(rows[:tw, :], rows[:tw, :], t1[:tw, :])
                        if nm == "k":
                            nc.sync.dma_start(rk_half[t0 : t0 + tw, :], rows[:tw, :])
                        for h in range(HG):
                            tp = tps.tile([128, 128], F32, tag="tps")
                            nc.tensor.transpose(
                                tp[:, :tw],
                                rows[:tw, h * 128 : (h + 1) * 128],
                                ident[:tw, :tw],
                            )
                            dst = kTm[h] if nm == "k" else qT[h]
                            nc.scalar.activation(
                                dst[:, t0 : t0 + tw], tp[:, :tw], AF.Copy
                            )
                for h in range(HG):
                    nc.sync.dma_start(
                        kT_in[h * 128 : (h + 1) * 128, :], kTm[h][:, :].bitcast(F32)
                    )
                nc.gpsimd.collective_compute(
                    "AllGather", ALU.bypass, replica_groups=G2,
                    ins=[kT_in[:]], outs=[kT_gath[:]],
                )

            # ================= attention =================
            with (
                tc.tile_pool(name="sps", bufs=2, space="PSUM") as spsp,
                tc.tile_pool(name="oacc", bufs=1, space="PSUM") as oaccp,
                tc.tile_pool(name="lacc", bufs=1, space="PSUM") as laccp,
                tc.tile_pool(name="kstream", bufs=4) as ksp,
                tc.tile_pool(name="pwork2", bufs=3) as pwp,
                tc.tile_pool(name="lwork", bufs=2) as lwp,
            ):
                for h in range(HG):
                    o_acc = oaccp.tile([128, SH], F32, tag="oacc")
                    l_acc = laccp.tile([1, SH], F32, tag="lacc")
                    nt = len(PTILES) + 2 * len(NTILES)
                    ki = 0
                    for src in range(3):  # 0 = past, 1..2 = gathered new blocks
                        tl = PTILES if src == 0 else NTILES
                        for t0, kw in tl:
                            kt_t = ksp.tile([128, 128], F32R, tag="kt")
                            v_t = ksp.tile([128, 128], F32R, tag="vt")
                            if src == 0:
                                nc.sync.dma_start(
                                    kt_t[:, :kw],
                                    pastkT[h * 128 : (h + 1) * 128, t0 : t0 + kw].bitcast(F32R),
                                )
                                nc.sync.dma_start(
                                    v_t[:kw, :],
                                    pastv[t0 : t0 + kw, h * 128 : (h + 1) * 128].bitcast(F32R),
                                )
                            else:
                                b = src - 1
                                nc.sync.dma_start(
                                    kt_t[:, :kw],
                                    kT_gath[b, h * 128 : (h + 1) * 128, t0 : t0 + kw].bitcast(F32R),
                                )
                                nc.sync.dma_start(
                                    v_t[:kw, :],
                                    v_gath[b, t0 : t0 + kw, h * 128 : (h + 1) * 128].bitcast(F32R),
                                )
                            s_ps = spsp.tile([128, SH], F32, tag="sps")
                            for g0, gw in QB:
                                nc.tensor.matmul(
                                    s_ps[:kw, g0 : g0 + gw],
                                    kt_t[:, :kw],
                                    qT[h][:, g0 : g0 + gw],
                                    start=True,
                                    stop=True,
                                )
                            p_t = pwp.tile([128, SH], F32R, tag="pt")
                            nc.scalar.activation(
                                p_t[:kw, :], s_ps[:kw, :], AF.Exp, scale=SCALE
                            )
                            first, last = ki == 0, ki == nt - 1
                            for g0, gw in QB:
                                nc.tensor.matmul(
                                    o_acc[:, g0 : g0 + gw],
                                    v_t[:kw, :],
                                    p_t[:kw, g0 : g0 + gw],
                                    start=first,
                                    stop=last,
                                )
                                nc.tensor.matmul(
                                    l_acc[:1, g0 : g0 + gw],
                                    ones_col[:kw, :1],
                                    p_t[:kw, g0 : g0 + gw],
                                    start=first,
                                    stop=last,
                                )
                            ki += 1
                    linv = lwp.tile([1, SH], F32, tag="linv")
                    nc.vector.reciprocal(linv[:1, :], l_acc[:1, :])
                    lbc = lwp.tile([128, SH], F32, tag="lbc")
                    nc.gpsimd.partition_broadcast(lbc[:, :], linv[:1, :])
                    nc.vector.tensor_mul(o_sb[h][:, :], o_acc[:, :], lbc[:, :])

            # ================= O projection =================
            with (
                tc.tile_pool(name="yps", bufs=2, space="PSUM") as ypp,
                tc.tile_pool(name="ywork", bufs=3) as ywp,
                tc.tile_pool(name="wo", bufs=1) as wop,
            ):
                wo = {}
                for hb in range(HG):
                    for nb in range(3):
                        t = wop.tile([128, 512], F32R, tag=f"wo{hb}{nb}", name=f"wo{hb}{nb}")
                        nc.sync.dma_start(
                            t[:],
                            woT[hb * 128 : (hb + 1) * 128, nb * 512 : (nb + 1) * 512].bitcast(F32R),
                        )
                        wo[hb, nb] = t
                for t0, tw in TOK:
                    for nb in range(3):
                        yp = ypp.tile([128, 512], F32, tag="yps")
                        for hb in range(HG):
                            nc.tensor.matmul(
                                yp[:tw, :],
                                o_sb[hb][:, t0 : t0 + tw],
                                wo[hb, nb][:, :],
                                start=(hb == 0),
                                stop=(hb == HG - 1),
                            )
                        ysb = ywp.tile([128, 512], F32, tag="ysb")
                        nc.vector.tensor_copy(ysb[:tw, :], yp[:tw, :])
                        nc.sync.dma_start(
                            y_part[t0 : t0 + tw, nb * 512 : (nb + 1) * 512], ysb[:tw, :]
                        )

    nc.compile()
    _BUILT["nc"] = nc
    return nc


def _host_prep(inputs):
    x = np.asarray(inputs["x"], dtype=np.float32).reshape(S, D)
    freqs = np.asarray(inputs["freqs"], dtype=np.float32)
    past_k = np.asarray(inputs["past_k"], dtype=np.float32).reshape(PAST, N, HD)
    past_v = np.asarray(inputs["past_v"], dtype=np.float32).reshape(PAST, N, HD)
    Wq = np.asarray(inputs["Wq"], dtype=np.float32)
    Wk = np.asarray(inputs["Wk"], dtype=np.float32)
    Wv = np.asarray(inputs["Wv"], dtype=np.float32)
    Wo = np.asarray(inputs["Wo"], dtype=np.float32)
    bq = np.asarray(inputs["bq"], dtype=np.float32)
    bk = np.asarray(inputs["bk"], dtype=np.float32)
    bv = np.asarray(inputs["bv"], dtype=np.float32)
    gq = np.asarray(inputs["gq"], dtype=np.float32)
    gk = np.asarray(inputs["gk"], dtype=np.float32)
    sf = int(np.asarray(inputs["start_frame"]))

    # rope angle table, matching reference._causal_rope
    c = HD // 2
    s0 = c - 2 * (c // 3)
    s1 = c // 3
    af = freqs[sf : sf + F, :s0]
    ah = freqs[:H, s0 : s0 + s1]
    aw = freqs[:W, s0 + s1 : s0 + 2 * s1]
    ang = np.concatenate(
        [
            np.broadcast_to(af[:, None, None, :], (F, H, W, s0)),
            np.broadcast_to(ah[None, :, None, :], (F, H, W, s1)),
            np.broadcast_to(aw[None, None, :, :], (F, H, W, s1)),
        ],
        axis=-1,
    ).reshape(S, c)
    cos = np.cos(ang).astype(np.float32)
    sin = np.sin(ang).astype(np.float32)
    # interleave pairs and tile across the 3 heads of a group
    cos_i = np.repeat(cos, 2, axis=1)               # [S, 128]
    sin_i = np.repeat(sin, 2, axis=1)
    sin_i[:, 0::2] *= -1.0                          # sign baked: even lane = -sin
    cos3 = np.tile(cos_i, (1, HG))                  # [S, 384]
    sin3 = np.tile(sin_i, (1, HG))

    xT_f = _round_f32r(x.T)                         # [D, S]
    pastkT_f = _round_f32r(
        past_k.transpose(1, 2, 0).reshape(N * HD, PAST)
    )                                               # [12*128, PAST]
    pastv_f = _round_f32r(past_v.reshape(PAST, N * HD))

    in_maps = []
    for core in range(NC_):
        i, j = core // 4, core % 4
        hsl = slice(j * HGD, (j + 1) * HGD)
        tsl = slice(i * SH, (i + 1) * SH)
        gq_j = gq[hsl]
        gk_j = gk[hsl]
        m = {
            "xT": np.ascontiguousarray(xT_f[:, tsl]),
            "wqT": _round_f32r(Wq.T[:, hsl]),
            "wkT": _round_f32r(Wk.T[:, hsl]),
            "wvT": _round_f32r(Wv.T[:, hsl]),
            "woT": _round_f32r(Wo.T[hsl, :]),
            "b3": _round_f32r(np.stack([bq[hsl], bk[hsl], bv[hsl]])),
            "cgq": np.ascontiguousarray(cos3[tsl] * gq_j[None, :]),
            "sgq": np.ascontiguousarray(sin3[tsl] * gq_j[None, :]),
            "cgk": np.ascontiguousarray(cos3[tsl] * gk_j[None, :]),
            "sgk": np.ascontiguousarray(sin3[tsl] * gk_j[None, :]),
            "pastkT": np.ascontiguousarray(pastkT_f[j * HGD : (j + 1) * HGD, :]),
            "pastv": np.ascontiguousarray(pastv_f[:, j * HGD : (j + 1) * HGD]),
        }
        in_maps.append(m)
    return in_maps, (bq, bk, bv, np.asarray(inputs["bo"], dtype=np.float32))


def kernel(**inputs):
    import os

    nc = _build()
    in_maps, (_, _, _, bo) = _host_prep(inputs)
    trace = bool(os.environ.get("KERNEL_TRACE"))
    kw = {}
    if trace:
        import tempfile

        kw = dict(trace=True, tmpdir=tempfile.mkdtemp(prefix="ktrace_"))
    res = run_bass_kernel_spmd(nc, in_maps, list(range(NC_)), **kw)
    if trace:
        print("HW exec time:", res.exec_time_ns, "ns")
        if res.instructions_and_trace is not None:
            print("trace path:", res.instructions_and_trace[1])

    y = np.zeros((S, D), dtype=np.float32)
    rk = np.zeros((S, N, HD), dtype=np.float32)
    v = np.zeros((S, N, HD), dtype=np.float32)
    for core in range(NC_):
        i, j = core // 4, core % 4
        r = res.results[core]
        y[i * SH : (i + 1) * SH] += r["y_part"]
        rk[i * SH : (i + 1) * SH, j * HG : (j + 1) * HG, :] = r["rk_half"].reshape(
            SH, HG, HD
        )
        v[i * SH : (i + 1) * SH, j * HG : (j + 1) * HG, :] = r["v_half"].reshape(
            SH, HG, HD
        )
    y += bo[None, :]
    return (
        y.reshape(B, S, D),
        rk.reshape(B, S, N, HD),
        v.reshape(B, S, N, HD),
    )


# revision 4
# speedup vs baseline: 1.1407x; 1.1407x over previous
"""Self-contained Trainium2 Bass kernel for CausalWanSelfAttention.

Sharding: 8 cores = 2 sequence-halves x 4 head-groups (3 heads each).
Per-core work: QKV projections for (its 780 tokens x its 384 dims), RMS
normalizers completed via one AllReduce of partial sum-of-squares, rope
on device, attention of (780 queries x 3 heads) against the full 7800-
position KV (past KV pre-tiled per head by the host; new KV exchanged
with the partner half via AllGather), then a partial O-projection. The
host sums the 4 head-group partials of y and stitches rk/v slices.

Numerics: projections / scores / O-projection in float32r (11-bit
mantissa, fp32 accumulate; DRAM operands pre-rounded on host so DMA'd
tiles are canonical fp32r). The softmax weights p and values v use bf16
for the PV matmul; the softmax denominator accumulates in fp32 PSUM via
ones-matmuls over pair-summed p tiles.
"""

import sys

if "/opt/trn_rl_repo" not in sys.path:
    sys.path.insert(0, "/opt/trn_rl_repo")

import math

import ml_dtypes
import numpy as np

import concourse.bass as bass
import concourse.mybir as mybir
import concourse.tile as tile
from concourse import bacc
from concourse.bass_utils import run_bass_kernel_spmd
from concourse.masks import make_identity

F32 = mybir.dt.float32
F32R = mybir.dt.float32r
BF16 = mybir.dt.bfloat16
AF = mybir.ActivationFunctionType
ALU = mybir.AluOpType

B, S, D, N, HD = 1, 1560, 1536, 12, 128
F, H, W = 1, 30, 52
PAST = 6240
EPS = 1e-6
NC_ = 8
SH = S // 2            # 780 tokens per sequence half
HG = 3                 # heads per group
HGD = HG * HD          # 384 dims per group
KD = D // 128          # 12 contraction tiles for projections
SCALE = 1.0 / math.sqrt(HD)

TOK = [(t, min(128, SH - t)) for t in range(0, SH, 128)]          # 6x128 + 12
QB = [(0, 512), (512, 268)]
NPT = (PAST + 127) // 128                                          # 49 past tiles
PT_KW = [min(128, PAST - t * 128) for t in range(NPT)]             # 48x128 + 96
NTILES = [(t, min(128, SH - t)) for t in range(0, SH, 128)]        # per new block

_BUILT = {}


def _round_f32r(x):
    xi = np.ascontiguousarray(x, dtype=np.float32).view(np.uint32)
    r = (xi + np.uint32(0x7FF) + ((xi >> np.uint32(12)) & np.uint32(1))) & np.uint32(
        0xFFFFF000
    )
    return r.view(np.float32)


def _build():
    if "nc" in _BUILT:
        return _BUILT["nc"]
    nc = bacc.Bacc("TRN2", target_bir_lowering=False, debug=False, num_devices=NC_)

    # ---- I/O ----
    xT = nc.dram_tensor("xT", [D, SH], F32, kind="ExternalInput")
    wqT = nc.dram_tensor("wqT", [D, HGD], F32, kind="ExternalInput")
    wkT = nc.dram_tensor("wkT", [D, HGD], F32, kind="ExternalInput")
    wvT = nc.dram_tensor("wvT", [D, HGD], F32, kind="ExternalInput")
    woT = nc.dram_tensor("woT", [HGD, D], F32, kind="ExternalInput")
    b3 = nc.dram_tensor("b3", [3, HGD], F32, kind="ExternalInput")
    cgq = nc.dram_tensor("cgq", [SH, HGD], F32, kind="ExternalInput")
    sgq = nc.dram_tensor("sgq", [SH, HGD], F32, kind="ExternalInput")
    cgk = nc.dram_tensor("cgk", [SH, HGD], F32, kind="ExternalInput")
    sgk = nc.dram_tensor("sgk", [SH, HGD], F32, kind="ExternalInput")
    pastkT = nc.dram_tensor("pastkT", [HG, NPT, 128, 128], F32, kind="ExternalInput")
    pastv = nc.dram_tensor("pastv", [HG, NPT, 128, 128], BF16, kind="ExternalInput")

    y_part = nc.dram_tensor("y_part", [SH, D], F32, kind="ExternalOutput")
    rk_half = nc.dram_tensor("rk_half", [SH, HGD], F32, kind="ExternalOutput")
    v_half = nc.dram_tensor("v_half", [SH, HGD], F32, kind="ExternalOutput")

    # ---- DRAM scratch for collectives ----
    ss_in = nc.dram_tensor("ss_in", [2, SH], F32)
    ss_out = nc.dram_tensor("ss_out", [2, SH], F32)
    kT_in = nc.dram_tensor("kT_in", [HGD, SH], F32)
    kT_gath = nc.dram_tensor("kT_gath", [2, HGD, SH], F32)
    v_in = nc.dram_tensor("v_in", [SH, HGD], BF16)
    v_gath = nc.dram_tensor("v_gath", [2, SH, HGD], BF16)

    G4 = [[0, 1, 2, 3], [4, 5, 6, 7]]
    G2 = [[0, 4], [1, 5], [2, 6], [3, 7]]

    with tile.TileContext(nc) as tc:
        with (
            tc.tile_pool(name="const", bufs=1) as constp,
            tc.tile_pool(name="resident", bufs=1) as resp,
        ):
            ident = constp.tile([128, 128], F32, tag="ident")
            make_identity(nc, ident)
            ones_f = constp.tile([128, 1], F32, tag="ones_f")
            nc.vector.memset(ones_f[:], 1.0)
            ones_bf = constp.tile([128, 1], BF16, tag="ones_bf")
            nc.vector.tensor_copy(ones_bf[:], ones_f[:])
            eps_t = constp.tile([128, 1], F32, tag="eps_t")
            nc.vector.memset(eps_t[:], EPS)
            onesr_f = constp.tile([1, 128], F32, tag="onesr_f")
            nc.vector.memset(onesr_f[:], 1.0)
            ones_row = constp.tile([1, 128], F32R, tag="ones_row")
            nc.vector.tensor_copy(ones_row[:], onesr_f[:])

            qT = [resp.tile([128, SH], F32R, tag=f"qT{h}", name=f"qT{h}") for h in range(HG)]
            o_sb = [resp.tile([128, SH], F32R, tag=f"osb{h}", name=f"osb{h}") for h in range(HG)]

            # ================= stage P =================
            with (
                tc.tile_pool(name="xw", bufs=1) as xwp,
                tc.tile_pool(name="projps", bufs=2, space="PSUM") as pps,
                tc.tile_pool(name="ptrans", bufs=2, space="PSUM") as tps,
                tc.tile_pool(name="pwork", bufs=3) as wkp,
                tc.tile_pool(name="praw", bufs=1) as rawp,
                tc.tile_pool(name="small", bufs=4) as smp,
            ):
                xt = []
                for kt in range(KD):
                    t = xwp.tile([128, SH], F32R, tag=f"x{kt}", name=f"x{kt}")
                    nc.sync.dma_start(t[:], xT[kt * 128 : (kt + 1) * 128, :].bitcast(F32R))
                    xt.append(t)
                wt = {}
                for nm, dram in (("k", wkT), ("q", wqT), ("v", wvT)):
                    for kt in range(KD):
                        t = xwp.tile([128, HGD], F32R, tag=f"w{nm}{kt}", name=f"w{nm}{kt}")
                        nc.sync.dma_start(
                            t[:], dram[kt * 128 : (kt + 1) * 128, :].bitcast(F32R)
                        )
                        wt[nm, kt] = t
                bias = {}
                for pi, nm in enumerate(("q", "k", "v")):
                    t = xwp.tile([1, HGD], F32R, tag=f"b{nm}", name=f"b{nm}")
                    nc.sync.dma_start(t[:], b3[pi : pi + 1, :].bitcast(F32R))
                    bias[nm] = t

                raws = {}
                # k and q interleaved so both partial sums land in ONE AllReduce
                for ti, (t0, tw) in enumerate(TOK):
                    psk = pps.tile([128, HGD], F32, tag="psk")
                    psq = pps.tile([128, HGD], F32, tag="psq")
                    for kt in range(KD):
                        nc.tensor.matmul(
                            psk[:tw, :], xt[kt][:, t0 : t0 + tw], wt["k", kt][:, :],
                            start=(kt == 0), stop=False,
                        )
                        nc.tensor.matmul(
                            psq[:tw, :], xt[kt][:, t0 : t0 + tw], wt["q", kt][:, :],
                            start=(kt == 0), stop=False,
                        )
                    nc.tensor.matmul(
                        psk[:tw, :], ones_row[:1, :tw], bias["k"][:1, :],
                        start=False, stop=True,
                    )
                    nc.tensor.matmul(
                        psq[:tw, :], ones_row[:1, :tw], bias["q"][:1, :],
                        start=False, stop=True,
                    )
                    for nm, ps, row in (("k", psk, 0), ("q", psq, 1)):
                        sq = wkp.tile([128, HGD], F32, tag="sqscratch")
                        ss = smp.tile([128, 1], F32, tag="ss")
                        nc.scalar.activation(
                            sq[:tw, :], ps[:tw, :], AF.Square, accum_out=ss[:tw, :]
                        )
                        nc.sync.dma_start(ss_in[row, t0 : t0 + tw], ss[:tw, 0:1])
                        raw = rawp.tile(
                            [128, HGD], F32, tag=f"raw_{nm}_{ti}", name=f"raw_{nm}_{ti}"
                        )
                        nc.vector.tensor_copy(raw[:tw, :], ps[:tw, :])
                        raws[nm, ti] = raw
                nc.gpsimd.collective_compute(
                    "AllReduce", ALU.add, replica_groups=G4,
                    ins=[ss_in[:]], outs=[ss_out[:]],
                )

                # V projection fills the AllReduce latency
                for ti, (t0, tw) in enumerate(TOK):
                    psv = pps.tile([128, HGD], F32, tag="psv")
                    for kt in range(KD):
                        nc.tensor.matmul(
                            psv[:tw, :], xt[kt][:, t0 : t0 + tw], wt["v", kt][:, :],
                            start=(kt == 0), stop=False,
                        )
                    nc.tensor.matmul(
                        psv[:tw, :], ones_row[:1, :tw], bias["v"][:1, :],
                        start=False, stop=True,
                    )
                    vrow = wkp.tile([128, HGD], F32, tag="vrow")
                    nc.vector.tensor_copy(vrow[:tw, :], psv[:tw, :])
                    nc.sync.dma_start(v_half[t0 : t0 + tw, :], vrow[:tw, :])
                    vr = wkp.tile([128, HGD], BF16, tag="vround")
                    nc.vector.tensor_copy(vr[:tw, :], vrow[:tw, :])
                    nc.sync.dma_start(v_in[t0 : t0 + tw, :], vr[:tw, :])
                nc.gpsimd.collective_compute(
                    "AllGather", ALU.bypass, replica_groups=G2,
                    ins=[v_in[:]], outs=[v_gath[:]],
                )

                # ---- rms + rope + transpose: k first (feeds gather), then q ----
                kTm = [
                    resp.tile([128, SH], F32R, tag=f"kTm{h}", name=f"kTm{h}")
                    for h in range(HG)
                ]
                for nm, row, cg_d, sg_d in (
                    ("k", 0, cgk, sgk),
                    ("q", 1, cgq, sgq),
                ):
                    for ti, (t0, tw) in enumerate(TOK):
                        sst = smp.tile([128, 1], F32, tag="sst")
                        nc.sync.dma_start(sst[:tw, 0:1], ss_out[row, t0 : t0 + tw])
                        sq2 = smp.tile([128, 1], F32, tag="sq2")
                        nc.scalar.activation(
                            sq2[:tw, :], sst[:tw, :], AF.Sqrt,
                            scale=1.0 / D, bias=eps_t[:tw, :],
                        )
                        rnorm = smp.tile([128, 1], F32, tag="rnorm")
                        nc.vector.reciprocal(rnorm[:tw, :], sq2[:tw, :])

                        cg_t = wkp.tile([128, HGD], F32, tag="cg")
                        nc.sync.dma_start(cg_t[:tw, :], cg_d[t0 : t0 + tw, :])
                        sg_t = wkp.tile([128, HGD], F32, tag="sg")
                        nc.sync.dma_start(sg_t[:tw, :], sg_d[t0 : t0 + tw, :])

                        raw = raws[nm, ti]
                        rv = raw[:tw, :].rearrange("p (c two) -> p c two", two=2)
                        swp = wkp.tile([128, HGD], F32, tag="swp")
                        sv = swp[:tw, :].rearrange("p (c two) -> p c two", two=2)
                        nc.vector.tensor_copy(sv[:, :, 0:1], rv[:, :, 1:2])
                        nc.vector.tensor_copy(sv[:, :, 1:2], rv[:, :, 0:1])
                        t1 = wkp.tile([128, HGD], F32, tag="ropet1")
                        nc.vector.scalar_tensor_tensor(
                            t1[:tw, :], raw[:tw, :], rnorm[:tw, :], cg_t[:tw, :],
                            ALU.mult, ALU.mult,
                        )
                        rows = wkp.tile([128, HGD], F32, tag="rows")
                        nc.vector.scalar_tensor_tensor(
                            rows[:tw, :], swp[:tw, :], rnorm[:tw, :], sg_t[:tw, :],
                            ALU.mult, ALU.mult,
                        )
                        nc.vector.tensor_add(rows[:tw, :], rows[:tw, :], t1[:tw, :])
                        if nm == "k":
                            nc.sync.dma_start(rk_half[t0 : t0 + tw, :], rows[:tw, :])
                        for h in range(HG):
                            tp = tps.tile([128, 128], F32, tag="tps")
                            nc.tensor.transpose(
                                tp[:, :tw],
                                rows[:tw, h * 128 : (h + 1) * 128],
                                ident[:tw, :tw],
                            )
                            dst = kTm[h] if nm == "k" else qT[h]
                            nc.scalar.activation(
                                dst[:, t0 : t0 + tw], tp[:, :tw], AF.Copy
                            )
                    if nm == "k":
                        for h in range(HG):
                            nc.sync.dma_start(
                                kT_in[h * 128 : (h + 1) * 128, :],
                                kTm[h][:, :].bitcast(F32),
                            )
                        nc.gpsimd.collective_compute(
                            "AllGather", ALU.bypass, replica_groups=G2,
                            ins=[kT_in[:]], outs=[kT_gath[:]],
                        )

            # ================= attention =================
            # kpos tiles: (src, tile-or-offset, kw). src 0 = past, 1/2 = new blocks
            klist = [(0, t, PT_KW[t]) for t in range(NPT)]
            for bidx in range(2):
                klist += [(1 + bidx, t0, kw) for t0, kw in NTILES]
            kws = [kw for _, _, kw in klist]
            # denominator plan: pair adjacent equal-kw tiles, singles otherwise
            plan = []
            pend = None
            for i, kw in enumerate(kws):
                if pend is not None and kws[pend] == kw:
                    plan.append(("pair", (pend, i), kw))
                    pend = None
                elif pend is not None:
                    plan.append(("single", (pend,), kws[pend]))
                    pend = i
                else:
                    pend = i
            if pend is not None:
                plan.append(("single", (pend,), kws[pend]))
            n_lmm = len(plan)
            trigger = {}
            for e in plan:
                trigger.setdefault(max(e[1]), []).append(e)

            with (
                tc.tile_pool(name="sps", bufs=2, space="PSUM") as spsp,
                tc.tile_pool(name="oacc", bufs=1, space="PSUM") as oaccp,
                tc.tile_pool(name="lacc", bufs=1, space="PSUM") as laccp,
                tc.tile_pool(name="kstream", bufs=4) as ksp,
                tc.tile_pool(name="pwork2", bufs=4) as pwp,
                tc.tile_pool(name="lwork", bufs=2) as lwp,
            ):
                for h in range(HG):
                    o_acc = oaccp.tile([128, SH], F32, tag="oacc")
                    l_acc = laccp.tile([1, SH], F32, tag="lacc")
                    li = 0

                    def emit_l(rhs, kw):
                        nonlocal li
                        for g0, gw in QB:
                            nc.tensor.matmul(
                                l_acc[:1, g0 : g0 + gw],
                                ones_bf[:kw, :1],
                                rhs[:kw, g0 : g0 + gw],
                                start=(li == 0),
                                stop=(li == n_lmm - 1),
                            )
                        li += 1

                    pstate = {}
                    prev = None
                    for idx, (src, tt, kw) in enumerate(klist):
                        kt_t = ksp.tile([128, 128], F32R, tag="kt")
                        v_t = ksp.tile([128, 128], BF16, tag="vt")
                        if src == 0:
                            nc.sync.dma_start(kt_t[:, :], pastkT[h, tt].bitcast(F32R))
                            nc.sync.dma_start(v_t[:, :], pastv[h, tt])
                        else:
                            bidx = src - 1
                            nc.sync.dma_start(
                                kt_t[:, :kw],
                                kT_gath[
                                    bidx, h * 128 : (h + 1) * 128, tt : tt + kw
                                ].bitcast(F32R),
                            )
                            nc.sync.dma_start(
                                v_t[:kw, :],
                                v_gath[bidx, tt : tt + kw, h * 128 : (h + 1) * 128],
                            )
                        s_ps = spsp.tile([128, SH], F32, tag="sps")
                        for g0, gw in QB:
                            nc.tensor.matmul(
                                s_ps[:kw, g0 : g0 + gw],
                                kt_t[:, :kw],
                                qT[h][:, g0 : g0 + gw],
                                start=True,
                                stop=True,
                            )
                        p_t = pwp.tile([128, SH], BF16, tag="pt")
                        nc.scalar.activation(p_t[:kw, :], s_ps[:kw, :], AF.Exp, scale=SCALE)
                        pstate[idx] = p_t

                        # lag-1 software pipeline: PV of the PREVIOUS tile
                        if prev is not None:
                            pv_t, pp_t, pkw, pfirst = prev
                            for g0, gw in QB:
                                nc.tensor.matmul(
                                    o_acc[:, g0 : g0 + gw],
                                    pv_t[:pkw, :],
                                    pp_t[:pkw, g0 : g0 + gw],
                                    start=pfirst,
                                    stop=False,
                                )
                        # denominator matmuls whose operands are now complete
                        for e in trigger.get(idx, []):
                            if e[0] == "pair":
                                i0, i1 = e[1]
                                p2 = pwp.tile([128, SH], BF16, tag="p2")
                                nc.vector.tensor_add(
                                    p2[: e[2], :], pstate[i0][: e[2], :], pstate[i1][: e[2], :]
                                )
                                emit_l(p2, e[2])
                            else:
                                emit_l(pstate[e[1][0]], e[2])
                        prev = (v_t, p_t, kw, idx == 0)
                    pv_t, pp_t, pkw, pfirst = prev
                    for g0, gw in QB:
                        nc.tensor.matmul(
                            o_acc[:, g0 : g0 + gw],
                            pv_t[:pkw, :],
                            pp_t[:pkw, g0 : g0 + gw],
                            start=pfirst,
                            stop=True,
                        )

                    linv = lwp.tile([1, SH], F32, tag="linv")
                    nc.vector.reciprocal(linv[:1, :], l_acc[:1, :])
                    lbc = lwp.tile([128, SH], F32, tag="lbc")
                    nc.gpsimd.partition_broadcast(lbc[:, :], linv[:1, :])
                    nc.vector.tensor_mul(o_sb[h][:, :], o_acc[:, :], lbc[:, :])

            # ================= O projection =================
            with (
                tc.tile_pool(name="yps", bufs=2, space="PSUM") as ypp,
                tc.tile_pool(name="ywork", bufs=3) as ywp,
                tc.tile_pool(name="wo", bufs=1) as wop,
            ):
                wo = {}
                for hb in range(HG):
                    for nb in range(3):
                        t = wop.tile(
                            [128, 512], F32R, tag=f"wo{hb}{nb}", name=f"wo{hb}{nb}"
                        )
                        nc.sync.dma_start(
                            t[:],
                            woT[
                                hb * 128 : (hb + 1) * 128, nb * 512 : (nb + 1) * 512
                            ].bitcast(F32R),
                        )
                        wo[hb, nb] = t
                for t0, tw in TOK:
                    for nb in range(3):
                        yp = ypp.tile([128, 512], F32, tag="yps")
                        for hb in range(HG):
                            nc.tensor.matmul(
                                yp[:tw, :],
                                o_sb[hb][:, t0 : t0 + tw],
                                wo[hb, nb][:, :],
                                start=(hb == 0),
                                stop=(hb == HG - 1),
                            )
                        ysb = ywp.tile([128, 512], F32, tag="ysb")
                        nc.vector.tensor_copy(ysb[:tw, :], yp[:tw, :])
                        nc.sync.dma_start(
                            y_part[t0 : t0 + tw, nb * 512 : (nb + 1) * 512], ysb[:tw, :]
                        )

    nc.compile()
    _BUILT["nc"] = nc
    return nc


def _host_prep(inputs):
    x = np.asarray(inputs["x"], dtype=np.float32).reshape(S, D)
    freqs = np.asarray(inputs["freqs"], dtype=np.float32)
    past_k = np.asarray(inputs["past_k"], dtype=np.float32).reshape(PAST, N, HD)
    past_v = np.asarray(inputs["past_v"], dtype=np.float32).reshape(PAST, N, HD)
    Wq = np.asarray(inputs["Wq"], dtype=np.float32)
    Wk = np.asarray(inputs["Wk"], dtype=np.float32)
    Wv = np.asarray(inputs["Wv"], dtype=np.float32)
    Wo = np.asarray(inputs["Wo"], dtype=np.float32)
    bq = np.asarray(inputs["bq"], dtype=np.float32)
    bk = np.asarray(inputs["bk"], dtype=np.float32)
    bv = np.asarray(inputs["bv"], dtype=np.float32)
    gq = np.asarray(inputs["gq"], dtype=np.float32)
    gk = np.asarray(inputs["gk"], dtype=np.float32)
    sf = int(np.asarray(inputs["start_frame"]))

    c = HD // 2
    s0 = c - 2 * (c // 3)
    s1 = c // 3
    af = freqs[sf : sf + F, :s0]
    ah = freqs[:H, s0 : s0 + s1]
    aw = freqs[:W, s0 + s1 : s0 + 2 * s1]
    ang = np.concatenate(
        [
            np.broadcast_to(af[:, None, None, :], (F, H, W, s0)),
            np.broadcast_to(ah[None, :, None, :], (F, H, W, s1)),
            np.broadcast_to(aw[None, None, :, :], (F, H, W, s1)),
        ],
        axis=-1,
    ).reshape(S, c)
    cos = np.cos(ang).astype(np.float32)
    sin = np.sin(ang).astype(np.float32)
    cos_i = np.repeat(cos, 2, axis=1)
    sin_i = np.repeat(sin, 2, axis=1)
    sin_i[:, 0::2] *= -1.0
    cos3 = np.tile(cos_i, (1, HG))
    sin3 = np.tile(sin_i, (1, HG))

    xT_f = _round_f32r(x.T)

    pkT = past_k.transpose(1, 2, 0)                     # [N, HD, PAST]
    pkT_t = np.zeros((N, NPT, 128, 128), dtype=np.float32)
    pvh = past_v.transpose(1, 0, 2)                     # [N, PAST, HD]
    pv_t = np.zeros((N, NPT, 128, 128), dtype=np.float32)
    for t in range(NPT):
        kw = PT_KW[t]
        pkT_t[:, t, :, :kw] = pkT[:, :, t * 128 : t * 128 + kw]
        pv_t[:, t, :kw, :] = pvh[:, t * 128 : t * 128 + kw, :]
    pkT_t = _round_f32r(pkT_t)
    pv_bf = pv_t.astype(ml_dtypes.bfloat16)

    in_maps = []
    for core in range(NC_):
        i, j = core // 4, core % 4
        hsl = slice(j * HGD, (j + 1) * HGD)
        tsl = slice(i * SH, (i + 1) * SH)
        gq_j = gq[hsl]
        gk_j = gk[hsl]
        m = {
            "xT": np.ascontiguousarray(xT_f[:, tsl]),
            "wqT": _round_f32r(Wq.T[:, hsl]),
            "wkT": _round_f32r(Wk.T[:, hsl]),
            "wvT": _round_f32r(Wv.T[:, hsl]),
            "woT": _round_f32r(Wo.T[hsl, :]),
            "b3": _round_f32r(np.stack([bq[hsl], bk[hsl], bv[hsl]])),
            "cgq": np.ascontiguousarray(cos3[tsl] * gq_j[None, :]),
            "sgq": np.ascontiguousarray(sin3[tsl] * gq_j[None, :]),
            "cgk": np.ascontiguousarray(cos3[tsl] * gk_j[None, :]),
            "sgk": np.ascontiguousarray(sin3[tsl] * gk_j[None, :]),
            "pastkT": np.ascontiguousarray(pkT_t[j * HG : (j + 1) * HG]),
            "pastv": np.ascontiguousarray(pv_bf[j * HG : (j + 1) * HG]),
        }
        in_maps.append(m)
    return in_maps, np.asarray(inputs["bo"], dtype=np.float32)


def kernel(**inputs):
    import os

    nc = _build()
    in_maps, bo = _host_prep(inputs)
    trace = bool(os.environ.get("KERNEL_TRACE"))
    kw = {}
    if trace:
        import tempfile

        kw = dict(trace=True, tmpdir=tempfile.mkdtemp(prefix="ktrace_"))
    res = run_bass_kernel_spmd(nc, in_maps, list(range(NC_)), **kw)
    if trace:
        print("HW exec time:", res.exec_time_ns, "ns")
        if res.instructions_and_trace is not None:
            print("trace path:", res.instructions_and_trace[1])

    y = np.zeros((S, D), dtype=np.float32)
    rk = np.zeros((S, N, HD), dtype=np.float32)
    v = np.zeros((S, N, HD), dtype=np.float32)
    for core in range(NC_):
        i, j = core // 4, core % 4
        r = res.results[core]
        y[i * SH : (i + 1) * SH] += r["y_part"]
        rk[i * SH : (i + 1) * SH, j * HG : (j + 1) * HG, :] = r["rk_half"].reshape(
            SH, HG, HD
        )
        v[i * SH : (i + 1) * SH, j * HG : (j + 1) * HG, :] = r["v_half"].reshape(
            SH, HG, HD
        )
    y += bo[None, :]
    return (
        y.reshape(B, S, D),
        rk.reshape(B, S, N, HD),
        v.reshape(B, S, N, HD),
    )


# revision 8
# speedup vs baseline: 1.1561x; 1.0135x over previous
"""Self-contained Trainium2 Bass kernel for CausalWanSelfAttention.

Sharding: 8 cores = 2 sequence-halves x 4 head-groups (3 heads each).
Per-core work: QKV projections for (its 780 tokens x its 384 dims), RMS
normalizers completed via one AllReduce of partial sum-of-squares, rope
on device, attention of (780 queries x 3 heads) against the full 7800-
position KV (past KV pre-tiled per head by the host; new KV exchanged
with the partner half via AllGather), then a partial O-projection. The
host sums the 4 head-group partials of y and stitches rk/v slices.

Numerics: projections / scores / O-projection in float32r (11-bit
mantissa, fp32 accumulate; DRAM operands pre-rounded on host so DMA'd
tiles are canonical fp32r). The softmax weights p and values v use bf16
for the PV matmul; the softmax denominator accumulates in fp32 PSUM via
ones-matmuls over pair-summed p tiles.
"""

import sys

if "/opt/trn_rl_repo" not in sys.path:
    sys.path.insert(0, "/opt/trn_rl_repo")

import math

import ml_dtypes
import numpy as np

import concourse.bass as bass
import concourse.mybir as mybir
import concourse.tile as tile
from concourse import bacc
from concourse.bass_utils import run_bass_kernel_spmd
from concourse.masks import make_identity

F32 = mybir.dt.float32
F32R = mybir.dt.float32r
F16 = mybir.dt.float16
AF = mybir.ActivationFunctionType
ALU = mybir.AluOpType

B, S, D, N, HD = 1, 1560, 1536, 12, 128
F, H, W = 1, 30, 52
PAST = 6240
EPS = 1e-6
NC_ = 8
SH = S // 2            # 780 tokens per sequence half
HG = 3                 # heads per group
HGD = HG * HD          # 384 dims per group
KD = D // 128          # 12 contraction tiles for projections
SCALE = 1.0 / math.sqrt(HD)

TOK = [(t, min(128, SH - t)) for t in range(0, SH, 128)]          # 6x128 + 12
QB = [(0, 512), (512, 268)]
NPT = (PAST + 127) // 128                                          # 49 past tiles
PT_KW = [min(128, PAST - t * 128) for t in range(NPT)]             # 48x128 + 96
NTILES = [(t, min(128, SH - t)) for t in range(0, SH, 128)]        # per new block

_BUILT = {}


def _round_f32r(x):
    xi = np.ascontiguousarray(x, dtype=np.float32).view(np.uint32)
    r = (xi + np.uint32(0x7FF) + ((xi >> np.uint32(12)) & np.uint32(1))) & np.uint32(
        0xFFFFF000
    )
    return r.view(np.float32)


def _build():
    if "nc" in _BUILT:
        return _BUILT["nc"]
    nc = bacc.Bacc("TRN2", target_bir_lowering=False, debug=False, num_devices=NC_)

    # ---- I/O ----
    xT = nc.dram_tensor("xT", [D, SH], F32, kind="ExternalInput")
    wqT = nc.dram_tensor("wqT", [D, HGD], F32, kind="ExternalInput")
    wkT = nc.dram_tensor("wkT", [D, HGD], F32, kind="ExternalInput")
    wvT = nc.dram_tensor("wvT", [D, HGD], F32, kind="ExternalInput")
    woT = nc.dram_tensor("woT", [HGD, D], F32, kind="ExternalInput")
    b3 = nc.dram_tensor("b3", [3, HGD], F32, kind="ExternalInput")
    cgq = nc.dram_tensor("cgq", [SH, HGD], F32, kind="ExternalInput")
    sgq = nc.dram_tensor("sgq", [SH, HGD], F32, kind="ExternalInput")
    cgk = nc.dram_tensor("cgk", [SH, HGD], F32, kind="ExternalInput")
    sgk = nc.dram_tensor("sgk", [SH, HGD], F32, kind="ExternalInput")
    pastkT = nc.dram_tensor("pastkT", [HG, NPT, 128, 128], F32, kind="ExternalInput")
    pastv = nc.dram_tensor("pastv", [HG, NPT, 128, 128], F16, kind="ExternalInput")

    y_part = nc.dram_tensor("y_part", [SH, D], F32, kind="ExternalOutput")
    rk_half = nc.dram_tensor("rk_half", [SH, HGD], F32, kind="ExternalOutput")
    v_half = nc.dram_tensor("v_half", [SH, HGD], F32, kind="ExternalOutput")

    # ---- DRAM scratch for collectives ----
    ss_in = nc.dram_tensor("ss_in", [2, SH], F32)
    ss_out = nc.dram_tensor("ss_out", [2, SH], F32)
    kT_in = nc.dram_tensor("kT_in", [HGD, SH], F32)
    kT_gath = nc.dram_tensor("kT_gath", [2, HGD, SH], F32)
    v_in = nc.dram_tensor("v_in", [SH, HGD], F16)
    v_gath = nc.dram_tensor("v_gath", [2, SH, HGD], F16)

    G4 = [[0, 1, 2, 3], [4, 5, 6, 7]]
    G2 = [[0, 4], [1, 5], [2, 6], [3, 7]]

    with tile.TileContext(nc) as tc:
        with (
            tc.tile_pool(name="const", bufs=1) as constp,
            tc.tile_pool(name="resident", bufs=1) as resp,
        ):
            ident = constp.tile([128, 128], F32, tag="ident")
            make_identity(nc, ident)
            ones_f = constp.tile([128, 1], F32, tag="ones_f")
            nc.vector.memset(ones_f[:], 1.0)
            ones_bf = constp.tile([128, 1], F16, tag="ones_hf")
            nc.vector.tensor_copy(ones_bf[:], ones_f[:])
            eps_t = constp.tile([128, 1], F32, tag="eps_t")
            nc.vector.memset(eps_t[:], EPS)
            onesr_f = constp.tile([1, 128], F32, tag="onesr_f")
            nc.vector.memset(onesr_f[:], 1.0)
            ones_row = constp.tile([1, 128], F32R, tag="ones_row")
            nc.vector.tensor_copy(ones_row[:], onesr_f[:])

            qT = [resp.tile([128, SH], F32R, tag=f"qT{h}", name=f"qT{h}") for h in range(HG)]
            o_sb = [resp.tile([128, SH], F32R, tag=f"osb{h}", name=f"osb{h}") for h in range(HG)]

            # ================= stage P =================
            with (
                tc.tile_pool(name="xw", bufs=1) as xwp,
                tc.tile_pool(name="projps", bufs=2, space="PSUM") as pps,
                tc.tile_pool(name="ptrans", bufs=2, space="PSUM") as tps,
                tc.tile_pool(name="pwork", bufs=3) as wkp,
                tc.tile_pool(name="praw", bufs=1) as rawp,
                tc.tile_pool(name="small", bufs=4) as smp,
            ):
                xt = []
                for kt in range(KD):
                    t = xwp.tile([128, SH], F32R, tag=f"x{kt}", name=f"x{kt}")
                    nc.sync.dma_start(t[:], xT[kt * 128 : (kt + 1) * 128, :].bitcast(F32R))
                    xt.append(t)
                wt = {}
                for nm, dram in (("k", wkT), ("q", wqT), ("v", wvT)):
                    for kt in range(KD):
                        t = xwp.tile([128, HGD], F32R, tag=f"w{nm}{kt}", name=f"w{nm}{kt}")
                        nc.sync.dma_start(
                            t[:], dram[kt * 128 : (kt + 1) * 128, :].bitcast(F32R)
                        )
                        wt[nm, kt] = t
                bias = {}
                for pi, nm in enumerate(("q", "k", "v")):
                    t = xwp.tile([1, HGD], F32R, tag=f"b{nm}", name=f"b{nm}")
                    nc.sync.dma_start(t[:], b3[pi : pi + 1, :].bitcast(F32R))
                    bias[nm] = t

                raws = {}
                # k and q interleaved so both partial sums land in ONE AllReduce
                for ti, (t0, tw) in enumerate(TOK):
                    psk = pps.tile([128, HGD], F32, tag="psk")
                    psq = pps.tile([128, HGD], F32, tag="psq")
                    for kt in range(KD):
                        nc.tensor.matmul(
                            psk[:tw, :], xt[kt][:, t0 : t0 + tw], wt["k", kt][:, :],
                            start=(kt == 0), stop=False,
                        )
                        nc.tensor.matmul(
                            psq[:tw, :], xt[kt][:, t0 : t0 + tw], wt["q", kt][:, :],
                            start=(kt == 0), stop=False,
                        )
                    nc.tensor.matmul(
                        psk[:tw, :], ones_row[:1, :tw], bias["k"][:1, :],
                        start=False, stop=True,
                    )
                    nc.tensor.matmul(
                        psq[:tw, :], ones_row[:1, :tw], bias["q"][:1, :],
                        start=False, stop=True,
                    )
                    for nm, ps, row in (("k", psk, 0), ("q", psq, 1)):
                        sq = wkp.tile([128, HGD], F32, tag="sqscratch")
                        ss = smp.tile([128, 1], F32, tag="ss")
                        nc.scalar.activation(
                            sq[:tw, :], ps[:tw, :], AF.Square, accum_out=ss[:tw, :]
                        )
                        nc.sync.dma_start(ss_in[row, t0 : t0 + tw], ss[:tw, 0:1])
                        raw = rawp.tile(
                            [128, HGD], F32, tag=f"raw_{nm}_{ti}", name=f"raw_{nm}_{ti}"
                        )
                        nc.vector.tensor_copy(raw[:tw, :], ps[:tw, :])
                        raws[nm, ti] = raw
                nc.gpsimd.collective_compute(
                    "AllReduce", ALU.add, replica_groups=G4,
                    ins=[ss_in[:]], outs=[ss_out[:]],
                )

                # V projection fills the AllReduce latency
                for ti, (t0, tw) in enumerate(TOK):
                    psv = pps.tile([128, HGD], F32, tag="psv")
                    for kt in range(KD):
                        nc.tensor.matmul(
                            psv[:tw, :], xt[kt][:, t0 : t0 + tw], wt["v", kt][:, :],
                            start=(kt == 0), stop=False,
                        )
                    nc.tensor.matmul(
                        psv[:tw, :], ones_row[:1, :tw], bias["v"][:1, :],
                        start=False, stop=True,
                    )
                    vrow = wkp.tile([128, HGD], F32, tag="vrow")
                    nc.vector.tensor_copy(vrow[:tw, :], psv[:tw, :])
                    nc.sync.dma_start(v_half[t0 : t0 + tw, :], vrow[:tw, :])
                    vr = wkp.tile([128, HGD], F16, tag="vround")
                    nc.vector.tensor_copy(vr[:tw, :], vrow[:tw, :])
                    nc.sync.dma_start(v_in[t0 : t0 + tw, :], vr[:tw, :])
                nc.gpsimd.collective_compute(
                    "AllGather", ALU.bypass, replica_groups=G2,
                    ins=[v_in[:]], outs=[v_gath[:]],
                )

                # ---- rms + rope + transpose: k first (feeds gather), then q ----
                kTm = [
                    resp.tile([128, SH], F32R, tag=f"kTm{h}", name=f"kTm{h}")
                    for h in range(HG)
                ]
                for nm, row, cg_d, sg_d in (
                    ("k", 0, cgk, sgk),
                    ("q", 1, cgq, sgq),
                ):
                    for ti, (t0, tw) in enumerate(TOK):
                        sst = smp.tile([128, 1], F32, tag="sst")
                        nc.sync.dma_start(sst[:tw, 0:1], ss_out[row, t0 : t0 + tw])
                        sq2 = smp.tile([128, 1], F32, tag="sq2")
                        nc.scalar.activation(
                            sq2[:tw, :], sst[:tw, :], AF.Sqrt,
                            scale=1.0 / D, bias=eps_t[:tw, :],
                        )
                        rnorm = smp.tile([128, 1], F32, tag="rnorm")
                        nc.vector.reciprocal(rnorm[:tw, :], sq2[:tw, :])

                        cg_t = wkp.tile([128, HGD], F32, tag="cg")
                        nc.sync.dma_start(cg_t[:tw, :], cg_d[t0 : t0 + tw, :])
                        sg_t = wkp.tile([128, HGD], F32, tag="sg")
                        nc.sync.dma_start(sg_t[:tw, :], sg_d[t0 : t0 + tw, :])

                        raw = raws[nm, ti]
                        rv = raw[:tw, :].rearrange("p (c two) -> p c two", two=2)
                        swp = wkp.tile([128, HGD], F32, tag="swp")
                        sv = swp[:tw, :].rearrange("p (c two) -> p c two", two=2)
                        nc.vector.tensor_copy(sv[:, :, 0:1], rv[:, :, 1:2])
                        nc.vector.tensor_copy(sv[:, :, 1:2], rv[:, :, 0:1])
                        t1 = wkp.tile([128, HGD], F32, tag="ropet1")
                        nc.vector.scalar_tensor_tensor(
                            t1[:tw, :], raw[:tw, :], rnorm[:tw, :], cg_t[:tw, :],
                            ALU.mult, ALU.mult,
                        )
                        rows = wkp.tile([128, HGD], F32, tag="rows")
                        nc.vector.scalar_tensor_tensor(
                            rows[:tw, :], swp[:tw, :], rnorm[:tw, :], sg_t[:tw, :],
                            ALU.mult, ALU.mult,
                        )
                        nc.vector.tensor_add(rows[:tw, :], rows[:tw, :], t1[:tw, :])
                        if nm == "k":
                            nc.sync.dma_start(rk_half[t0 : t0 + tw, :], rows[:tw, :])
                        for h in range(HG):
                            tp = tps.tile([128, 128], F32, tag="tps")
                            nc.tensor.transpose(
                                tp[:, :tw],
                                rows[:tw, h * 128 : (h + 1) * 128],
                                ident[:tw, :tw],
                            )
                            dst = kTm[h] if nm == "k" else qT[h]
                            nc.scalar.activation(
                                dst[:, t0 : t0 + tw], tp[:, :tw], AF.Copy
                            )
                    if nm == "k":
                        for h in range(HG):
                            nc.sync.dma_start(
                                kT_in[h * 128 : (h + 1) * 128, :],
                                kTm[h][:, :].bitcast(F32),
                            )
                        nc.gpsimd.collective_compute(
                            "AllGather", ALU.bypass, replica_groups=G2,
                            ins=[kT_in[:]], outs=[kT_gath[:]],
                        )

            # ================= attention =================
            # kpos tiles: (src, tile-or-offset, kw). src 0 = past, 1/2 = new blocks
            klist = [(0, t, PT_KW[t]) for t in range(NPT)]
            for bidx in range(2):
                klist += [(1 + bidx, t0, kw) for t0, kw in NTILES]
            kws = [kw for _, _, kw in klist]
            # denominator plan: pair adjacent equal-kw tiles, singles otherwise
            plan = []
            pend = None
            for i, kw in enumerate(kws):
                if pend is not None and kws[pend] == kw:
                    plan.append(("pair", (pend, i), kw))
                    pend = None
                elif pend is not None:
                    plan.append(("single", (pend,), kws[pend]))
                    pend = i
                else:
                    pend = i
            if pend is not None:
                plan.append(("single", (pend,), kws[pend]))
            n_lmm = len(plan)
            trigger = {}
            for e in plan:
                trigger.setdefault(max(e[1]), []).append(e)

            with (
                tc.tile_pool(name="sps", bufs=2, space="PSUM") as spsp,
                tc.tile_pool(name="oacc", bufs=1, space="PSUM") as oaccp,
                tc.tile_pool(name="lacc", bufs=1, space="PSUM") as laccp,
                tc.tile_pool(name="kstream", bufs=4) as ksp,
                tc.tile_pool(name="pwork2", bufs=4) as pwp,
                tc.tile_pool(name="lwork", bufs=2) as lwp,
            ):
                for h in range(HG):
                    o_acc = oaccp.tile([128, SH], F32, tag="oacc")
                    l_acc = laccp.tile([1, SH], F32, tag="lacc")
                    li = 0

                    def emit_l(rhs, kw):
                        nonlocal li
                        for g0, gw in QB:
                            nc.tensor.matmul(
                                l_acc[:1, g0 : g0 + gw],
                                ones_bf[:kw, :1],
                                rhs[:kw, g0 : g0 + gw],
                                start=(li == 0),
                                stop=(li == n_lmm - 1),
                            )
                        li += 1

                    pstate = {}
                    prev = None
                    for idx, (src, tt, kw) in enumerate(klist):
                        kt_t = ksp.tile([128, 128], F32R, tag="kt")
                        v_t = ksp.tile([128, 128], F16, tag="vt")
                        if src == 0:
                            nc.sync.dma_start(kt_t[:, :], pastkT[h, tt].bitcast(F32R))
                            nc.sync.dma_start(v_t[:, :], pastv[h, tt])
                        else:
                            bidx = src - 1
                            nc.sync.dma_start(
                                kt_t[:, :kw],
                                kT_gath[
                                    bidx, h * 128 : (h + 1) * 128, tt : tt + kw
                                ].bitcast(F32R),
                            )
                            nc.sync.dma_start(
                                v_t[:kw, :],
                                v_gath[bidx, tt : tt + kw, h * 128 : (h + 1) * 128],
                            )
                        s_ps = spsp.tile([128, SH], F32, tag="sps")
                        for g0, gw in QB:
                            nc.tensor.matmul(
                                s_ps[:kw, g0 : g0 + gw],
                                kt_t[:, :kw],
                                qT[h][:, g0 : g0 + gw],
                                start=True,
                                stop=True,
                            )
                        p_t = pwp.tile([128, SH], F16, tag="pt")
                        nc.scalar.activation(p_t[:kw, :], s_ps[:kw, :], AF.Exp, scale=SCALE)
                        pstate[idx] = p_t

                        # lag-1 software pipeline: PV of the PREVIOUS tile
                        if prev is not None:
                            pv_t, pp_t, pkw, pfirst = prev
                            for g0, gw in QB:
                                nc.tensor.matmul(
                                    o_acc[:, g0 : g0 + gw],
                                    pv_t[:pkw, :],
                                    pp_t[:pkw, g0 : g0 + gw],
                                    start=pfirst,
                                    stop=False,
                                )
                        # denominator matmuls whose operands are now complete
                        for e in trigger.get(idx, []):
                            if e[0] == "pair":
                                i0, i1 = e[1]
                                p2 = pwp.tile([128, SH], F16, tag="p2")
                                nc.vector.tensor_add(
                                    p2[: e[2], :], pstate[i0][: e[2], :], pstate[i1][: e[2], :]
                                )
                                emit_l(p2, e[2])
                            else:
                                emit_l(pstate[e[1][0]], e[2])
                        prev = (v_t, p_t, kw, idx == 0)
                    pv_t, pp_t, pkw, pfirst = prev
                    for g0, gw in QB:
                        nc.tensor.matmul(
                            o_acc[:, g0 : g0 + gw],
                            pv_t[:pkw, :],
                            pp_t[:pkw, g0 : g0 + gw],
                            start=pfirst,
                            stop=True,
                        )

                    linv = lwp.tile([1, SH], F32, tag="linv")
                    nc.vector.reciprocal(linv[:1, :], l_acc[:1, :])
                    lbc = lwp.tile([128, SH], F32, tag="lbc")
                    nc.gpsimd.partition_broadcast(lbc[:, :], linv[:1, :])
                    nc.vector.tensor_mul(o_sb[h][:, :], o_acc[:, :], lbc[:, :])

            # ================= O projection =================
            with (
                tc.tile_pool(name="yps", bufs=2, space="PSUM") as ypp,
                tc.tile_pool(name="ywork", bufs=3) as ywp,
                tc.tile_pool(name="wo", bufs=1) as wop,
            ):
                wo = {}
                for hb in range(HG):
                    for nb in range(3):
                        t = wop.tile(
                            [128, 512], F32R, tag=f"wo{hb}{nb}", name=f"wo{hb}{nb}"
                        )
                        nc.sync.dma_start(
                            t[:],
                            woT[
                                hb * 128 : (hb + 1) * 128, nb * 512 : (nb + 1) * 512
                            ].bitcast(F32R),
                        )
                        wo[hb, nb] = t
                for t0, tw in TOK:
                    for nb in range(3):
                        yp = ypp.tile([128, 512], F32, tag="yps")
                        for hb in range(HG):
                            nc.tensor.matmul(
                                yp[:tw, :],
                                o_sb[hb][:, t0 : t0 + tw],
                                wo[hb, nb][:, :],
                                start=(hb == 0),
                                stop=(hb == HG - 1),
                            )
                        ysb = ywp.tile([128, 512], F32, tag="ysb")
                        nc.vector.tensor_copy(ysb[:tw, :], yp[:tw, :])
                        nc.sync.dma_start(
                            y_part[t0 : t0 + tw, nb * 512 : (nb + 1) * 512], ysb[:tw, :]
                        )

    nc.compile()
    _BUILT["nc"] = nc
    return nc


def _host_prep(inputs):
    x = np.asarray(inputs["x"], dtype=np.float32).reshape(S, D)
    freqs = np.asarray(inputs["freqs"], dtype=np.float32)
    past_k = np.asarray(inputs["past_k"], dtype=np.float32).reshape(PAST, N, HD)
    past_v = np.asarray(inputs["past_v"], dtype=np.float32).reshape(PAST, N, HD)
    Wq = np.asarray(inputs["Wq"], dtype=np.float32)
    Wk = np.asarray(inputs["Wk"], dtype=np.float32)
    Wv = np.asarray(inputs["Wv"], dtype=np.float32)
    Wo = np.asarray(inputs["Wo"], dtype=np.float32)
    bq = np.asarray(inputs["bq"], dtype=np.float32)
    bk = np.asarray(inputs["bk"], dtype=np.float32)
    bv = np.asarray(inputs["bv"], dtype=np.float32)
    gq = np.asarray(inputs["gq"], dtype=np.float32)
    gk = np.asarray(inputs["gk"], dtype=np.float32)
    sf = int(np.asarray(inputs["start_frame"]))

    c = HD // 2
    s0 = c - 2 * (c // 3)
    s1 = c // 3
    af = freqs[sf : sf + F, :s0]
    ah = freqs[:H, s0 : s0 + s1]
    aw = freqs[:W, s0 + s1 : s0 + 2 * s1]
    ang = np.concatenate(
        [
            np.broadcast_to(af[:, None, None, :], (F, H, W, s0)),
            np.broadcast_to(ah[None, :, None, :], (F, H, W, s1)),
            np.broadcast_to(aw[None, None, :, :], (F, H, W, s1)),
        ],
        axis=-1,
    ).reshape(S, c)
    cos = np.cos(ang).astype(np.float32)
    sin = np.sin(ang).astype(np.float32)
    cos_i = np.repeat(cos, 2, axis=1)
    sin_i = np.repeat(sin, 2, axis=1)
    sin_i[:, 0::2] *= -1.0
    cos3 = np.tile(cos_i, (1, HG))
    sin3 = np.tile(sin_i, (1, HG))

    xT_f = _round_f32r(x.T)

    pkT = past_k.transpose(1, 2, 0)                     # [N, HD, PAST]
    pkT_t = np.zeros((N, NPT, 128, 128), dtype=np.float32)
    pvh = past_v.transpose(1, 0, 2)                     # [N, PAST, HD]
    pv_t = np.zeros((N, NPT, 128, 128), dtype=np.float32)
    for t in range(NPT):
        kw = PT_KW[t]
        pkT_t[:, t, :, :kw] = pkT[:, :, t * 128 : t * 128 + kw]
        pv_t[:, t, :kw, :] = pvh[:, t * 128 : t * 128 + kw, :]
    pkT_t = _round_f32r(pkT_t)
    pv_bf = pv_t.astype(np.float16)

    in_maps = []
    for core in range(NC_):
        i, j = core // 4, core % 4
        hsl = slice(j * HGD, (j + 1) * HGD)
        tsl = slice(i * SH, (i + 1) * SH)
        gq_j = gq[hsl]
        gk_j = gk[hsl]
        m = {
            "xT": np.ascontiguousarray(xT_f[:, tsl]),
            "wqT": _round_f32r(Wq.T[:, hsl]),
            "wkT": _round_f32r(Wk.T[:, hsl]),
            "wvT": _round_f32r(Wv.T[:, hsl]),
            "woT": _round_f32r(Wo.T[hsl, :]),
            "b3": _round_f32r(np.stack([bq[hsl], bk[hsl], bv[hsl]])),
            "cgq": np.ascontiguousarray(cos3[tsl] * gq_j[None, :]),
            "sgq": np.ascontiguousarray(sin3[tsl] * gq_j[None, :]),
            "cgk": np.ascontiguousarray(cos3[tsl] * gk_j[None, :]),
            "sgk": np.ascontiguousarray(sin3[tsl] * gk_j[None, :]),
            "pastkT": np.ascontiguousarray(pkT_t[j * HG : (j + 1) * HG]),
            "pastv": np.ascontiguousarray(pv_bf[j * HG : (j + 1) * HG]),
        }
        in_maps.append(m)
    return in_maps, np.asarray(inputs["bo"], dtype=np.float32)


def kernel(**inputs):
    import os

    nc = _build()
    in_maps, bo = _host_prep(inputs)
    trace = bool(os.environ.get("KERNEL_TRACE"))
    kw = {}
    if trace:
        import tempfile

        kw = dict(trace=True, tmpdir=tempfile.mkdtemp(prefix="ktrace_"))
    res = run_bass_kernel_spmd(nc, in_maps, list(range(NC_)), **kw)
    if trace:
        print("HW exec time:", res.exec_time_ns, "ns")
        if res.instructions_and_trace is not None:
            print("trace path:", res.instructions_and_trace[1])

    y = np.zeros((S, D), dtype=np.float32)
    rk = np.zeros((S, N, HD), dtype=np.float32)
    v = np.zeros((S, N, HD), dtype=np.float32)
    for core in range(NC_):
        i, j = core // 4, core % 4
        r = res.results[core]
        y[i * SH : (i + 1) * SH] += r["y_part"]
        rk[i * SH : (i + 1) * SH, j * HG : (j + 1) * HG, :] = r["rk_half"].reshape(
            SH, HG, HD
        )
        v[i * SH : (i + 1) * SH, j * HG : (j + 1) * HG, :] = r["v_half"].reshape(
            SH, HG, HD
        )
    y += bo[None, :]
    return (
        y.reshape(B, S, D),
        rk.reshape(B, S, N, HD),
        v.reshape(B, S, N, HD),
    )
